# revision 13
# baseline (speedup 1.0000x reference)
"""CausalPointNetEncoder on 8 TRN2 NeuronCores (Bass/Tile, SPMD data-parallel).

Strategy
--------
Shard the 1024 polylines 128-per-core. Activations are kept feature-major
([feat partitions, rows free]) so chained matmuls need no transposes
(out_T = W^T @ x_T), BN scale/bias are per-partition ops, and the causal
cummax runs along the free dim via the DVE tensor_tensor_scan instruction
(segmented across polyline boundaries with a mask operand).

BatchNorm uses training-mode batch stats over ALL 262144 rows, so each
layer needs one tiny cross-core AllReduce of [128,2] partial sums
(sum, sum-of-squares). Per-core stats are NOT acceptable (8.7% error).

Algebraic folds used:
 - b0..b3 cancel inside BN (mean subtraction absorbs them).
 - sign(g_k) is folded into W_k columns host-side so s_k = |g_k|*rstd >= 0.
 - x_k = s_k * max(raw_k + c1_k, 0) with c1_k = be_k/s_k - mean(raw_k);
   the s_k scale is folded into the NEXT layer's weight rows on device,
   so the BN apply is a single 2-op DVE tensor_scalar (add, max) at 4x.
 - cummax commutes with (+c, max0, *s>=0): pooled = s*max(cummax(raw)+c1,0),
   so layer 2 needs no separate evacuation pass (the scan reads PSUM).

Host does layout only: transpose input to feature-major, transpose output
back to row-major, dtype conversion of weights to bf16.
"""

import numpy as np

import concourse.bass as bass
import concourse.mybir as mybir
from concourse.tile import TileContext
from concourse.bass_utils import run_bass_kernel_spmd

try:
    import ml_dtypes

    BF16 = np.float16
except ImportError:  # pragma: no cover
    import jax.numpy as jnp

    BF16 = jnp.bfloat16

# problem sizes (hardcoded per harness contract)
B, A, T, C, H, O = 16, 64, 256, 32, 128, 64
N_CORES = 8
BA = B * A                      # 1024 polylines
P_CORE = BA // N_CORES          # 128 polylines per core
R = P_CORE * T                  # 32768 rows per core
N_TOTAL = BA * T                # 262144 rows globally
EPS = 1e-5

CH = 2048                       # rows per chunk (8 polylines)
NCH = R // CH                   # 16 chunks
PS = 1024                       # psum tile free size (2 banks)
NEG = -60000.0  # fp16-safe segment sentinel

F32 = mybir.dt.float32
F32R = mybir.dt.float32r
BF = mybir.dt.float16
ALU = mybir.AluOpType
AFT = mybir.ActivationFunctionType


def _split_multi_waits(nc):
    """This walrus build accepts at most ONE sync wait per instruction.
    Hoist all but the last wait of each instruction onto same-engine NoOp
    carriers inserted immediately before it (engine queues execute in
    program order, so ordering is preserved)."""
    cnt = 0
    for f in nc.m.functions:
        for bb in f.blocks:
            il = bb.instructions
            if not any(i.sync_info and len(i.sync_info.on_wait) > 1 for i in il):
                continue
            new = []
            for inst in il:
                si = inst.sync_info
                waits = list(si.on_wait) if si else []
                if len(waits) > 1:
                    for w in waits[:-1]:
                        nop = mybir.InstNoOp(name=f"I-wsplit-{cnt}", ins=[], outs=[])
                        cnt += 1
                        nop.engine = inst.engine
                        nop.sync_info = mybir.SyncInfo(on_wait=[w], on_update=[])
                        new.append(nop)
                    inst.sync_info = mybir.SyncInfo(
                        on_wait=[waits[-1]], on_update=list(si.on_update)
                    )
                new.append(inst)
            bb.instructions = new
    return cnt


def build_nc():
    nc = bass.Bass()

    xin = nc.declare_dram_parameter("xfm", [C, R], F32R, isOutput=False)
    w0 = nc.declare_dram_parameter("w0", [C, H], F32R, isOutput=False)
    w1t = nc.declare_dram_parameter("w1t", [H, H], BF, isOutput=False)
    w1b = nc.declare_dram_parameter("w1b", [H, H], BF, isOutput=False)
    w2 = nc.declare_dram_parameter("w2", [H, H], BF, isOutput=False)
    w3 = nc.declare_dram_parameter("w3", [H, H], BF, isOutput=False)
    w4 = nc.declare_dram_parameter("w4", [H, O], BF, isOutput=False)
    gabs = nc.declare_dram_parameter("gabs", [H, 4], F32, isOutput=False)
    bes = nc.declare_dram_parameter("bes", [H, 4], F32, isOutput=False)
    b4 = nc.declare_dram_parameter("b4", [O, 1], F32, isOutput=False)
    yout = nc.declare_dram_parameter("out", [O, R], F32, isOutput=True)

    # collective bounce buffers (internal DRAM)
    parts = [nc.dram_tensor(f"part{k}", [H, 2], F32) for k in range(4)]
    reds = [nc.dram_tensor(f"red{k}", [H, 2], F32) for k in range(4)]

    with TileContext(nc) as tc:
        with (
            tc.tile_pool(name="sing", bufs=1) as sing,
            tc.tile_pool(name="big", bufs=1) as big,
            tc.tile_pool(name="inp", bufs=2) as inp,
            tc.tile_pool(name="roll", bufs=3) as roll,
            tc.tile_pool(name="roll2", bufs=3) as roll2,
            tc.tile_pool(name="scr", bufs=2) as scrp,
            tc.tile_pool(name="stat", bufs=1) as stat,
            tc.tile_pool(name="ps", bufs=3, space="PSUM") as psp,
            tc.tile_pool(name="pss", bufs=1, space="PSUM") as pss,
        ):
            # ---- weights / constants ----
            w0_sb = sing.tile([C, H], F32R, tag="w0")
            nc.sync.dma_start(out=w0_sb, in_=w0[:])
            w1t_sb = sing.tile([H, H], BF, tag="w1t")
            nc.sync.dma_start(out=w1t_sb, in_=w1t[:])
            w1b_sb = sing.tile([H, H], BF, tag="w1b")
            nc.sync.dma_start(out=w1b_sb, in_=w1b[:])
            w2_sb = sing.tile([H, H], BF, tag="w2")
            nc.sync.dma_start(out=w2_sb, in_=w2[:])
            w3_sb = sing.tile([H, H], BF, tag="w3")
            nc.sync.dma_start(out=w3_sb, in_=w3[:])
            w4_sb = sing.tile([H, O], BF, tag="w4")
            nc.sync.dma_start(out=w4_sb, in_=w4[:])
            gabs_sb = sing.tile([H, 4], F32, tag="gabs")
            nc.sync.dma_start(out=gabs_sb, in_=gabs[:])
            bes_sb = sing.tile([H, 4], F32, tag="bes")
            nc.sync.dma_start(out=bes_sb, in_=bes[:])
            b4_sb = sing.tile([O, 1], F32, tag="b4")
            nc.sync.dma_start(out=b4_sb, in_=b4[:])

            # scaled weight copies (filled after each barrier)
            w1t_s = sing.tile([H, H], BF, tag="w1t_s")
            w1b_s = sing.tile([H, H], BF, tag="w1b_s")
            w2_s = sing.tile([H, H], BF, tag="w2_s")
            w3_s = sing.tile([H, H], BF, tag="w3_s")
            w4_s = sing.tile([H, O], BF, tag="w4_s")

            # scan masks
            mask01 = sing.tile([H, CH], BF, tag="mask01")
            nc.vector.memset(mask01, 1.0)
            nc.vector.memset(
                mask01.rearrange("p (n t) -> p n t", t=T)[:, :, 0:1], 0.0
            )
            maskneg = sing.tile([H, PS], BF, tag="maskneg")
            nc.vector.memset(maskneg, 0.0)
            nc.vector.memset(
                maskneg.rearrange("p (n t) -> p n t", t=T)[:, :, 0:1], NEG
            )

            # staged activations (bf16): A and Cbuf are reused across layers
            preA = big.tile([H, R], BF, tag="bigA")   # pre0; later cmax2
            preC = big.tile([H, R], BF, tag="bigC")   # pre1; later pre3

            # accumulators: one column per accum op instance
            accS = [stat.tile([H, 2 * NCH], F32, name=f"accS{k}", tag=f"accS{k}") for k in range(4)]
            accQ = [stat.tile([H, 2 * NCH], F32, name=f"accQ{k}", tag=f"accQ{k}") for k in range(4)]
            for t_ in accS + accQ:
                nc.vector.memset(t_, 0.0)

            # per-layer stat vectors
            c1 = [stat.tile([H, 1], F32, name=f"c1_{k}", tag=f"c1_{k}") for k in range(4)]
            svec = [stat.tile([H, 1], F32, name=f"s_{k}", tag=f"s_{k}") for k in range(4)]
            tmp1 = stat.tile([H, 1], F32, tag="tmp1")
            tmp2 = stat.tile([H, 1], F32, tag="tmp2")
            tmp3 = stat.tile([H, 1], F32, tag="tmp3")
            gpart = stat.tile([H, 2], F32, tag="gpart")
            eps_sb = stat.tile([H, 1], F32, tag="eps")
            nc.vector.memset(eps_sb, EPS)
            gstat = [stat.tile([H, 2], F32, name=f"gstat{k}", tag=f"gstat{k}") for k in range(4)]

            def barrier(k, w_scale_jobs):
                """Reduce local accums -> allreduce -> finalize s_k, c1_k,
                scale next-layer weights."""
                nc.vector.tensor_reduce(
                    gpart[:, 0:1], accS[k][:], mybir.AxisListType.X, ALU.add
                )
                nc.vector.tensor_reduce(
                    gpart[:, 1:2], accQ[k][:], mybir.AxisListType.X, ALU.add
                )
                nc.sync.dma_start(out=parts[k][:], in_=gpart)
                nc.gpsimd.collective_compute(
                    "AllReduce",
                    ALU.add,
                    replica_groups=[list(range(N_CORES))],
                    ins=[parts[k][:]],
                    outs=[reds[k][:]],
                )
                nc.sync.dma_start(out=gstat[k], in_=reds[k][:])
                g = gstat[k]
                sumv, sumq = g[:, 0:1], g[:, 1:2]
                if k == 2:
                    # sum(raw2) = W2s^T @ sum(x1~)  (tiny fp32 matmul, FD=1)
                    w2_sf = stat.tile([H, H], F32, tag="w2_sf")
                    nc.vector.tensor_copy(w2_sf, w2_s)
                    ps1 = pss.tile([H, 1], F32, tag="stats_ps")
                    nc.tensor.matmul(ps1, lhsT=w2_sf, rhs=sumv, start=True, stop=True)
                    nc.vector.tensor_copy(tmp3, ps1)
                    sumv = tmp3
                # mu = sum/N ; m2 = sumsq/N ; var = m2 - mu^2
                mu = tmp1
                nc.vector.tensor_scalar(mu, sumv, 1.0 / N_TOTAL, None, ALU.mult)
                nc.vector.tensor_scalar(tmp2, sumq, 1.0 / N_TOTAL, None, ALU.mult)
                var = tmp2
                musq = stat.tile([H, 1], F32, tag="musq")
                nc.vector.tensor_tensor(musq, mu, mu, ALU.mult)
                nc.vector.tensor_tensor(var, var, musq, ALU.subtract)
                # s = |g| / sqrt(var + eps)
                std = stat.tile([H, 1], F32, tag="std")
                nc.scalar.activation(std, var, AFT.Sqrt, bias=eps_sb, scale=1.0)
                rstd = stat.tile([H, 1], F32, tag="rstd")
                nc.vector.reciprocal(rstd, std)
                nc.vector.tensor_tensor(svec[k], rstd, gabs_sb[:, k : k + 1], ALU.mult)
                nc.vector.tensor_scalar(svec[k], svec[k], 1e-20, None, ALU.max)
                # c1 = be/s - mu
                recs = stat.tile([H, 1], F32, tag="recs")
                nc.vector.reciprocal(recs, svec[k])
                nc.vector.tensor_tensor(c1[k], bes_sb[:, k : k + 1], recs, ALU.mult)
                nc.vector.tensor_tensor(c1[k], c1[k], mu, ALU.subtract)
                # fold s_k into next-layer weight rows
                for wdst, wsrc in w_scale_jobs:
                    nc.vector.tensor_scalar(wdst, wsrc, svec[k], None, ALU.mult)

            # ================= phase 0: mm0 + evac0 + sumsq0 ==============
            for ci in range(NCH):
                cs = ci * CH
                xt = inp.tile([C, CH], F32R, tag="xin")
                nc.sync.dma_start(out=xt, in_=xin[:, cs : cs + CH])
                for h in range(2):
                    pt = psp.tile([H, PS], F32, tag="mmps")
                    for q in range(2):
                        nc.tensor.matmul(
                            pt[:, q * 512 : (q + 1) * 512],
                            lhsT=w0_sb,
                            rhs=xt[:, h * PS + q * 512 : h * PS + (q + 1) * 512],
                            start=True,
                            stop=True,
                        )
                    dst = preA[:, cs + h * PS : cs + (h + 1) * PS]
                    nc.scalar.activation(
                        dst, pt, AFT.Copy, accum_out=accS[0][:, 2 * ci + h : 2 * ci + h + 1]
                    )
                    scr = scrp.tile([H, PS], BF, tag="scr")
                    nc.scalar.activation(
                        scr, pt, AFT.Square,
                        accum_out=accQ[0][:, 2 * ci + h : 2 * ci + h + 1],
                    )
            barrier(0, [(w1t_s, w1t_sb), (w1b_s, w1b_sb)])

            # ============ phase 1: apply0 + scan0 + mm1 + evac1/sumsq1 ====
            for ci in range(NCH):
                cs = ci * CH
                x0 = roll.tile([H, CH], BF, tag="xroll")
                nc.vector.tensor_scalar(
                    x0, preA[:, cs : cs + CH], c1[0], 0.0, ALU.add, ALU.max
                )
                p0 = roll2.tile([H, CH], BF, tag="r2roll")
                nc.vector.tensor_tensor_scan(
                    p0, mask01, x0, 0.0, ALU.mult, ALU.max
                )
                for h in range(2):
                    pt = psp.tile([H, PS], F32, tag="mmps")
                    for q in range(2):
                        sl = slice(h * PS + q * 512, h * PS + (q + 1) * 512)
                        nc.tensor.matmul(
                            pt[:, q * 512 : (q + 1) * 512],
                            lhsT=w1t_s, rhs=x0[:, sl], start=True, stop=False,
                        )
                        nc.tensor.matmul(
                            pt[:, q * 512 : (q + 1) * 512],
                            lhsT=w1b_s, rhs=p0[:, sl], start=False, stop=True,
                        )
                    dst = preC[:, cs + h * PS : cs + (h + 1) * PS]
                    nc.scalar.activation(
                        dst, pt, AFT.Copy, accum_out=accS[1][:, 2 * ci + h : 2 * ci + h + 1]
                    )
                    scr = scrp.tile([H, PS], BF, tag="scr")
                    nc.vector.scalar_tensor_tensor(
                        scr,
                        pt,
                        1.0,
                        dst,
                        ALU.mult,
                        ALU.mult,
                        accum_out=accQ[1][:, 2 * ci + h : 2 * ci + h + 1],
                    )
            barrier(1, [(w2_s, w2_sb)])

            # ============ phase 2: apply1 + mm2 + scan2(PSUM) + sumsq2 ====
            for ci in range(NCH):
                cs = ci * CH
                x1 = roll.tile([H, CH], BF, tag="xroll")
                # NB: tensor_scalar+accum_out is broken on this HW path
                # (probe: accum garbage AND corrupt main out) — use ACT Relu.
                nc.scalar.activation(
                    x1, preC[:, cs : cs + CH], AFT.Relu, bias=c1[1], scale=1.0,
                    accum_out=accS[2][:, 2 * ci : 2 * ci + 1],
                )
                for h in range(2):
                    pt = psp.tile([H, PS], F32, tag="mmps")
                    for q in range(2):
                        sl = slice(h * PS + q * 512, h * PS + (q + 1) * 512)
                        nc.tensor.matmul(
                            pt[:, q * 512 : (q + 1) * 512],
                            lhsT=w2_s, rhs=x1[:, sl], start=True, stop=True,
                        )
                    # scan evacuates PSUM -> cmax2 (reuses preA storage)
                    nc.vector.tensor_tensor_scan(
                        preA[:, cs + h * PS : cs + (h + 1) * PS],
                        maskneg, pt, NEG, ALU.add, ALU.max,
                    )
                    scr2 = scrp.tile([H, PS], BF, tag="scr")
                    nc.scalar.activation(
                        scr2,
                        pt,
                        AFT.Square,
                        accum_out=accQ[2][:, 2 * ci + h : 2 * ci + h + 1],
                    )
            barrier(2, [(w3_s, w3_sb)])

            # ============ phase 3: p2-apply + mm3 + evac3/sumsq3 ==========
            for ci in range(NCH):
                cs = ci * CH
                p2 = roll.tile([H, CH], BF, tag="xroll")
                nc.vector.tensor_scalar(
                    p2, preA[:, cs : cs + CH], c1[2], 0.0, ALU.add, ALU.max
                )
                for h in range(2):
                    pt = psp.tile([H, PS], F32, tag="mmps")
                    for q in range(2):
                        sl = slice(h * PS + q * 512, h * PS + (q + 1) * 512)
                        nc.tensor.matmul(
                            pt[:, q * 512 : (q + 1) * 512],
                            lhsT=w3_s, rhs=p2[:, sl], start=True, stop=True,
                        )
                    dst = preC[:, cs + h * PS : cs + (h + 1) * PS]
                    nc.scalar.activation(
                        dst, pt, AFT.Copy, accum_out=accS[3][:, 2 * ci + h : 2 * ci + h + 1]
                    )
                    scr = scrp.tile([H, PS], BF, tag="scr")
                    nc.vector.scalar_tensor_tensor(
                        scr,
                        pt,
                        1.0,
                        dst,
                        ALU.mult,
                        ALU.mult,
                        accum_out=accQ[3][:, 2 * ci + h : 2 * ci + h + 1],
                    )
            barrier(3, [(w4_s, w4_sb)])

            # ============ phase 4: apply3 + mm4 + bias + out ==============
            for ci in range(NCH):
                cs = ci * CH
                x3 = roll.tile([H, CH], BF, tag="xroll")
                nc.vector.tensor_scalar(
                    x3, preC[:, cs : cs + CH], c1[3], 0.0, ALU.add, ALU.max
                )
                for h in range(2):
                    pt = psp.tile([H, PS], F32, tag="mmps")
                    for q in range(2):
                        sl = slice(h * PS + q * 512, h * PS + (q + 1) * 512)
                        nc.tensor.matmul(
                            pt[:O, q * 512 : (q + 1) * 512],
                            lhsT=w4_s, rhs=x3[:, sl], start=True, stop=True,
                        )
                    ot = roll2.tile([O, PS], F32, tag="r2roll")
                    nc.scalar.activation(
                        ot, pt[:O, :], AFT.Identity, bias=b4_sb, scale=1.0
                    )
                    nc.sync.dma_start(
                        out=yout[:, cs + h * PS : cs + (h + 1) * PS], in_=ot
                    )

    _split_multi_waits(nc)
    return nc


_NC_CACHE = None


def kernel(**inputs):
    global _NC_CACHE
    pl = np.asarray(inputs["polylines"], np.float32).reshape(BA, T, C)
    W0 = np.asarray(inputs["W0"], np.float32)
    W1 = np.asarray(inputs["W1"], np.float32)
    W2 = np.asarray(inputs["W2"], np.float32)
    W3 = np.asarray(inputs["W3"], np.float32)
    W4 = np.asarray(inputs["W4"], np.float32)
    b4v = np.asarray(inputs["b4"], np.float32)
    g = [np.asarray(inputs[f"g{k}"], np.float32) for k in range(4)]
    be = [np.asarray(inputs[f"be{k}"], np.float32) for k in range(4)]

    # host-side sign folding: W_k columns *= sign(g_k) so s_k >= 0 on device
    sg = [np.where(gk < 0, -1.0, 1.0).astype(np.float32) for gk in g]
    W0f = W0 * sg[0][None, :]
    W1tf = (W1[:H] * sg[1][None, :]).astype(BF16)
    W1bf = (W1[H:] * sg[1][None, :]).astype(BF16)
    W2f = (W2 * sg[2][None, :]).astype(BF16)
    W3f = (W3 * sg[3][None, :]).astype(BF16)
    W4f = W4.astype(BF16)
    gabs_np = np.stack([np.abs(gk) for gk in g], 1).astype(np.float32)  # [H,4]
    bes_np = np.stack(be, 1).astype(np.float32)                        # [H,4]

    shared = {
        "w0": W0f,
        "w1t": np.ascontiguousarray(W1tf),
        "w1b": np.ascontiguousarray(W1bf),
        "w2": np.ascontiguousarray(W2f),
        "w3": np.ascontiguousarray(W3f),
        "w4": np.ascontiguousarray(W4f),
        "gabs": gabs_np,
        "bes": bes_np,
        "b4": b4v.reshape(O, 1),
    }
    in_maps = []
    for i in range(N_CORES):
        rows = pl[i * P_CORE : (i + 1) * P_CORE].reshape(R, C)
        xfm = np.ascontiguousarray(rows.T)  # [C, R] feature-major
        in_maps.append({"xfm": xfm, **shared})

    if _NC_CACHE is None:
        _NC_CACHE = build_nc()
    res = run_bass_kernel_spmd(_NC_CACHE, in_maps, list(range(N_CORES)))

    outs = []
    for i in range(N_CORES):
        o = np.asarray(res.results[i]["out"])  # [O, R] feature-major
        outs.append(o.T.reshape(P_CORE, T, O))
    full = np.concatenate(outs, 0)  # [BA, T, O]
    return full.reshape(B, A, T, O).astype(np.float32)


# revision 14
# speedup vs baseline: 1.1144x; 1.1144x over previous
"""CausalPointNetEncoder on 8 TRN2 NeuronCores (Bass/Tile, SPMD data-parallel).

Strategy
--------
Shard the 1024 polylines 128-per-core. Activations are kept feature-major
([feat partitions, rows free]) so chained matmuls need no transposes
(out_T = W^T @ x_T), BN scale/bias are per-partition ops, and the causal
cummax runs along the free dim via the DVE tensor_tensor_scan instruction
(segmented across polyline boundaries with a mask operand).

BatchNorm uses training-mode batch stats over ALL 262144 rows, so each
layer needs one tiny cross-core AllReduce of [128,2] partial sums
(sum, sum-of-squares). Per-core stats are NOT acceptable (8.7% error).

Algebraic folds used:
 - b0..b3 cancel inside BN (mean subtraction absorbs them).
 - sign(g_k) is folded into W_k columns host-side so s_k = |g_k|*rstd >= 0.
 - x_k = s_k * max(raw_k + c1_k, 0) with c1_k = be_k/s_k - mean(raw_k);
   the s_k scale is folded into the NEXT layer's weight rows on device,
   so the BN apply is a single 2-op DVE tensor_scalar (add, max) at 4x.
 - cummax commutes with (+c, max0, *s>=0): pooled = s*max(cummax(raw)+c1,0),
   so layer 2 needs no separate evacuation pass (the scan reads PSUM).

Host does layout only: transpose input to feature-major, transpose output
back to row-major, dtype conversion of weights to bf16.
"""

import numpy as np

import concourse.bass as bass
import concourse.mybir as mybir
from concourse.tile import TileContext
from concourse.bass_utils import run_bass_kernel_spmd

try:
    import ml_dtypes

    BF16 = np.float16
except ImportError:  # pragma: no cover
    import jax.numpy as jnp

    BF16 = jnp.bfloat16

# problem sizes (hardcoded per harness contract)
B, A, T, C, H, O = 16, 64, 256, 32, 128, 64
N_CORES = 8
BA = B * A                      # 1024 polylines
P_CORE = BA // N_CORES          # 128 polylines per core
R = P_CORE * T                  # 32768 rows per core
N_TOTAL = BA * T                # 262144 rows globally
EPS = 1e-5

CH = 2048                       # rows per chunk (8 polylines)
NCH = R // CH                   # 16 chunks
PS = 1024                       # psum tile free size (2 banks)
NEG = -60000.0  # fp16-safe segment sentinel

F32 = mybir.dt.float32
F32R = mybir.dt.float32r
BF = mybir.dt.float16
ALU = mybir.AluOpType
AFT = mybir.ActivationFunctionType


def _split_multi_waits(nc):
    """This walrus build accepts at most ONE sync wait per instruction.
    Hoist all but the last wait of each instruction onto same-engine NoOp
    carriers inserted immediately before it (engine queues execute in
    program order, so ordering is preserved)."""
    cnt = 0
    for f in nc.m.functions:
        for bb in f.blocks:
            il = bb.instructions
            if not any(i.sync_info and len(i.sync_info.on_wait) > 1 for i in il):
                continue
            new = []
            for inst in il:
                si = inst.sync_info
                waits = list(si.on_wait) if si else []
                if len(waits) > 1:
                    for w in waits[:-1]:
                        nop = mybir.InstNoOp(name=f"I-wsplit-{cnt}", ins=[], outs=[])
                        cnt += 1
                        nop.engine = inst.engine
                        nop.sync_info = mybir.SyncInfo(on_wait=[w], on_update=[])
                        new.append(nop)
                    inst.sync_info = mybir.SyncInfo(
                        on_wait=[waits[-1]], on_update=list(si.on_update)
                    )
                new.append(inst)
            bb.instructions = new
    return cnt


def build_nc():
    nc = bass.Bass()

    xin = nc.declare_dram_parameter("xfm", [C, R], F32R, isOutput=False)
    w0 = nc.declare_dram_parameter("w0", [C, H], F32R, isOutput=False)
    w1t = nc.declare_dram_parameter("w1t", [H, H], BF, isOutput=False)
    w1b = nc.declare_dram_parameter("w1b", [H, H], BF, isOutput=False)
    w2 = nc.declare_dram_parameter("w2", [H, H], BF, isOutput=False)
    w3 = nc.declare_dram_parameter("w3", [H, H], BF, isOutput=False)
    w4 = nc.declare_dram_parameter("w4", [H, O], BF, isOutput=False)
    gabs = nc.declare_dram_parameter("gabs", [H, 4], F32, isOutput=False)
    bes = nc.declare_dram_parameter("bes", [H, 4], F32, isOutput=False)
    b4 = nc.declare_dram_parameter("b4", [O, 1], F32, isOutput=False)
    yout = nc.declare_dram_parameter("out", [O, R], F32, isOutput=True)

    # collective bounce buffers (internal DRAM)
    parts = [nc.dram_tensor(f"part{k}", [H, 2], F32) for k in range(4)]
    reds = [nc.dram_tensor(f"red{k}", [H, 2], F32) for k in range(4)]
    warm_p = nc.dram_tensor("warm_p", [H, 2], F32)
    warm_r = nc.dram_tensor("warm_r", [H, 2], F32)

    with TileContext(nc) as tc:
        with (
            tc.tile_pool(name="sing", bufs=1) as sing,
            tc.tile_pool(name="big", bufs=1) as big,
            tc.tile_pool(name="inp", bufs=2) as inp,
            tc.tile_pool(name="roll", bufs=3) as roll,
            tc.tile_pool(name="roll2", bufs=3) as roll2,
            tc.tile_pool(name="scr", bufs=2) as scrp,
            tc.tile_pool(name="stat", bufs=1) as stat,
            tc.tile_pool(name="ps", bufs=3, space="PSUM") as psp,
            tc.tile_pool(name="pss", bufs=1, space="PSUM") as pss,
        ):
            # ---- weights / constants ----
            w0_sb = sing.tile([C, H], F32R, tag="w0")
            nc.sync.dma_start(out=w0_sb, in_=w0[:])
            w1t_sb = sing.tile([H, H], BF, tag="w1t")
            nc.sync.dma_start(out=w1t_sb, in_=w1t[:])
            w1b_sb = sing.tile([H, H], BF, tag="w1b")
            nc.sync.dma_start(out=w1b_sb, in_=w1b[:])
            w2_sb = sing.tile([H, H], BF, tag="w2")
            nc.sync.dma_start(out=w2_sb, in_=w2[:])
            w3_sb = sing.tile([H, H], BF, tag="w3")
            nc.sync.dma_start(out=w3_sb, in_=w3[:])
            w4_sb = sing.tile([H, O], BF, tag="w4")
            nc.sync.dma_start(out=w4_sb, in_=w4[:])
            gabs_sb = sing.tile([H, 4], F32, tag="gabs")
            nc.sync.dma_start(out=gabs_sb, in_=gabs[:])
            bes_sb = sing.tile([H, 4], F32, tag="bes")
            nc.sync.dma_start(out=bes_sb, in_=bes[:])
            b4_sb = sing.tile([O, 1], F32, tag="b4")
            nc.sync.dma_start(out=b4_sb, in_=b4[:])

            # scaled weight copies (filled after each barrier)
            w1t_s = sing.tile([H, H], BF, tag="w1t_s")
            w1b_s = sing.tile([H, H], BF, tag="w1b_s")
            w2_s = sing.tile([H, H], BF, tag="w2_s")
            w3_s = sing.tile([H, H], BF, tag="w3_s")
            w4_s = sing.tile([H, O], BF, tag="w4_s")

            # scan masks
            mask01 = sing.tile([H, CH], BF, tag="mask01")
            nc.vector.memset(mask01, 1.0)
            nc.vector.memset(
                mask01.rearrange("p (n t) -> p n t", t=T)[:, :, 0:1], 0.0
            )
            maskneg = sing.tile([H, PS], BF, tag="maskneg")
            nc.vector.memset(maskneg, 0.0)
            nc.vector.memset(
                maskneg.rearrange("p (n t) -> p n t", t=T)[:, :, 0:1], NEG
            )

            # warmup collective: pays the cold CC handshake during phase 0
            wtile = stat.tile([H, 2], F32, tag="wtile")
            nc.vector.memset(wtile, 0.0)
            nc.sync.dma_start(out=warm_p[:], in_=wtile)
            nc.gpsimd.collective_compute(
                "AllReduce", ALU.add,
                replica_groups=[list(range(N_CORES))],
                ins=[warm_p[:]], outs=[warm_r[:]],
            )

            # staged activations (bf16): A and Cbuf are reused across layers
            preA = big.tile([H, R], BF, tag="bigA")   # pre0; later cmax2
            preC = big.tile([H, R], BF, tag="bigC")   # pre1; later pre3

            # accumulators: one column per accum op instance
            accS = [stat.tile([H, 2 * NCH], F32, name=f"accS{k}", tag=f"accS{k}") for k in range(4)]
            accQ = [stat.tile([H, 2 * NCH], F32, name=f"accQ{k}", tag=f"accQ{k}") for k in range(4)]
            for t_ in accS + accQ:
                nc.vector.memset(t_, 0.0)

            # per-layer stat vectors
            c1 = [stat.tile([H, 1], F32, name=f"c1_{k}", tag=f"c1_{k}") for k in range(4)]
            svec = [stat.tile([H, 1], F32, name=f"s_{k}", tag=f"s_{k}") for k in range(4)]
            tmp1 = stat.tile([H, 1], F32, tag="tmp1")
            tmp2 = stat.tile([H, 1], F32, tag="tmp2")
            tmp3 = stat.tile([H, 1], F32, tag="tmp3")
            gpart = stat.tile([H, 2], F32, tag="gpart")
            eps_sb = stat.tile([H, 1], F32, tag="eps")
            nc.vector.memset(eps_sb, EPS)
            gstat = [stat.tile([H, 2], F32, name=f"gstat{k}", tag=f"gstat{k}") for k in range(4)]

            def barrier(k, w_scale_jobs):
                """Reduce local accums -> allreduce -> finalize s_k, c1_k,
                scale next-layer weights."""
                nc.vector.tensor_reduce(
                    gpart[:, 0:1], accS[k][:], mybir.AxisListType.X, ALU.add
                )
                nc.vector.tensor_reduce(
                    gpart[:, 1:2], accQ[k][:], mybir.AxisListType.X, ALU.add
                )
                nc.sync.dma_start(out=parts[k][:], in_=gpart)
                nc.gpsimd.collective_compute(
                    "AllReduce",
                    ALU.add,
                    replica_groups=[list(range(N_CORES))],
                    ins=[parts[k][:]],
                    outs=[reds[k][:]],
                )
                nc.sync.dma_start(out=gstat[k], in_=reds[k][:])
                g = gstat[k]
                sumv, sumq = g[:, 0:1], g[:, 1:2]
                if k == 2:
                    # sum(raw2) = W2s^T @ sum(x1~)  (tiny fp32 matmul, FD=1)
                    w2_sf = stat.tile([H, H], F32, tag="w2_sf")
                    nc.vector.tensor_copy(w2_sf, w2_s)
                    ps1 = pss.tile([H, 1], F32, tag="stats_ps")
                    nc.tensor.matmul(ps1, lhsT=w2_sf, rhs=sumv, start=True, stop=True)
                    nc.vector.tensor_copy(tmp3, ps1)
                    sumv = tmp3
                # mu = sum/N ; m2 = sumsq/N ; var = m2 - mu^2
                mu = tmp1
                nc.vector.tensor_scalar(mu, sumv, 1.0 / N_TOTAL, None, ALU.mult)
                nc.vector.tensor_scalar(tmp2, sumq, 1.0 / N_TOTAL, None, ALU.mult)
                var = tmp2
                musq = stat.tile([H, 1], F32, tag="musq")
                nc.vector.tensor_tensor(musq, mu, mu, ALU.mult)
                nc.vector.tensor_tensor(var, var, musq, ALU.subtract)
                # s = |g| / sqrt(var + eps)
                std = stat.tile([H, 1], F32, tag="std")
                nc.scalar.activation(std, var, AFT.Sqrt, bias=eps_sb, scale=1.0)
                rstd = stat.tile([H, 1], F32, tag="rstd")
                nc.vector.reciprocal(rstd, std)
                nc.vector.tensor_tensor(svec[k], rstd, gabs_sb[:, k : k + 1], ALU.mult)
                nc.vector.tensor_scalar(svec[k], svec[k], 1e-20, None, ALU.max)
                # c1 = be/s - mu
                recs = stat.tile([H, 1], F32, tag="recs")
                nc.vector.reciprocal(recs, svec[k])
                nc.vector.tensor_tensor(c1[k], bes_sb[:, k : k + 1], recs, ALU.mult)
                nc.vector.tensor_tensor(c1[k], c1[k], mu, ALU.subtract)
                # fold s_k into next-layer weight rows
                for wdst, wsrc in w_scale_jobs:
                    nc.vector.tensor_scalar(wdst, wsrc, svec[k], None, ALU.mult)

            # ================= phase 0: mm0 + evac0 + sumsq0 ==============
            for ci in range(NCH):
                cs = ci * CH
                xt = inp.tile([C, CH], F32R, tag="xin")
                nc.sync.dma_start(out=xt, in_=xin[:, cs : cs + CH])
                for h in range(2):
                    pt = psp.tile([H, PS], F32, tag="mmps")
                    for q in range(2):
                        nc.tensor.matmul(
                            pt[:, q * 512 : (q + 1) * 512],
                            lhsT=w0_sb,
                            rhs=xt[:, h * PS + q * 512 : h * PS + (q + 1) * 512],
                            start=True,
                            stop=True,
                        )
                    dst = preA[:, cs + h * PS : cs + (h + 1) * PS]
                    nc.scalar.activation(
                        dst, pt, AFT.Copy, accum_out=accS[0][:, 2 * ci + h : 2 * ci + h + 1]
                    )
                    scr = scrp.tile([H, PS], BF, tag="scr")
                    nc.vector.scalar_tensor_tensor(
                        scr, pt, 1.0, dst, ALU.mult, ALU.mult,
                        accum_out=accQ[0][:, 2 * ci + h : 2 * ci + h + 1],
                    )
            barrier(0, [(w1t_s, w1t_sb), (w1b_s, w1b_sb)])

            # ============ phase 1: apply0 + scan0 + mm1 + evac1/sumsq1 ====
            for ci in range(NCH):
                cs = ci * CH
                x0 = roll.tile([H, CH], BF, tag="xroll")
                nc.vector.tensor_scalar(
                    x0, preA[:, cs : cs + CH], c1[0], 0.0, ALU.add, ALU.max
                )
                p0 = roll2.tile([H, CH], BF, tag="r2roll")
                nc.vector.tensor_tensor_scan(
                    p0, mask01, x0, 0.0, ALU.mult, ALU.max
                )
                for h in range(2):
                    pt = psp.tile([H, PS], F32, tag="mmps")
                    for q in range(2):
                        sl = slice(h * PS + q * 512, h * PS + (q + 1) * 512)
                        nc.tensor.matmul(
                            pt[:, q * 512 : (q + 1) * 512],
                            lhsT=w1t_s, rhs=x0[:, sl], start=True, stop=False,
                        )
                        nc.tensor.matmul(
                            pt[:, q * 512 : (q + 1) * 512],
                            lhsT=w1b_s, rhs=p0[:, sl], start=False, stop=True,
                        )
                    dst = preC[:, cs + h * PS : cs + (h + 1) * PS]
                    nc.scalar.activation(
                        dst, pt, AFT.Copy, accum_out=accS[1][:, 2 * ci + h : 2 * ci + h + 1]
                    )
                    scr = scrp.tile([H, PS], BF, tag="scr")
                    nc.scalar.activation(
                        scr, pt, AFT.Square,
                        accum_out=accQ[1][:, 2 * ci + h : 2 * ci + h + 1],
                    )
            barrier(1, [(w2_s, w2_sb)])

            # ============ phase 2: apply1 + mm2 + scan2(PSUM) + sumsq2 ====
            for ci in range(NCH):
                cs = ci * CH
                x1 = roll.tile([H, CH], BF, tag="xroll")
                # NB: tensor_scalar+accum_out is broken on this HW path
                # (probe: accum garbage AND corrupt main out) — use ACT Relu.
                nc.scalar.activation(
                    x1, preC[:, cs : cs + CH], AFT.Relu, bias=c1[1], scale=1.0,
                    accum_out=accS[2][:, 2 * ci : 2 * ci + 1],
                )
                for h in range(2):
                    pt = psp.tile([H, PS], F32, tag="mmps")
                    for q in range(2):
                        sl = slice(h * PS + q * 512, h * PS + (q + 1) * 512)
                        nc.tensor.matmul(
                            pt[:, q * 512 : (q + 1) * 512],
                            lhsT=w2_s, rhs=x1[:, sl], start=True, stop=True,
                        )
                    # scan evacuates PSUM -> cmax2 (reuses preA storage)
                    nc.vector.tensor_tensor_scan(
                        preA[:, cs + h * PS : cs + (h + 1) * PS],
                        maskneg, pt, NEG, ALU.add, ALU.max,
                    )
                    scr2 = scrp.tile([H, PS], BF, tag="scr")
                    nc.scalar.activation(
                        scr2,
                        pt,
                        AFT.Square,
                        accum_out=accQ[2][:, 2 * ci + h : 2 * ci + h + 1],
                    )
            barrier(2, [(w3_s, w3_sb)])

            # ============ phase 3: p2-apply + mm3 + evac3/sumsq3 ==========
            for ci in range(NCH):
                cs = ci * CH
                p2 = roll.tile([H, CH], BF, tag="xroll")
                nc.vector.tensor_scalar(
                    p2, preA[:, cs : cs + CH], c1[2], 0.0, ALU.add, ALU.max
                )
                for h in range(2):
                    pt = psp.tile([H, PS], F32, tag="mmps")
                    for q in range(2):
                        sl = slice(h * PS + q * 512, h * PS + (q + 1) * 512)
                        nc.tensor.matmul(
                            pt[:, q * 512 : (q + 1) * 512],
                            lhsT=w3_s, rhs=p2[:, sl], start=True, stop=True,
                        )
                    dst = preC[:, cs + h * PS : cs + (h + 1) * PS]
                    nc.scalar.activation(
                        dst, pt, AFT.Copy, accum_out=accS[3][:, 2 * ci + h : 2 * ci + h + 1]
                    )
                    scr = scrp.tile([H, PS], BF, tag="scr")
                    nc.vector.scalar_tensor_tensor(
                        scr,
                        pt,
                        1.0,
                        dst,
                        ALU.mult,
                        ALU.mult,
                        accum_out=accQ[3][:, 2 * ci + h : 2 * ci + h + 1],
                    )
            barrier(3, [(w4_s, w4_sb)])

            # ============ phase 4: apply3 + mm4 + bias + out ==============
            for ci in range(NCH):
                cs = ci * CH
                x3 = roll.tile([H, CH], BF, tag="xroll")
                nc.vector.tensor_scalar(
                    x3, preC[:, cs : cs + CH], c1[3], 0.0, ALU.add, ALU.max
                )
                for h in range(2):
                    pt = psp.tile([H, PS], F32, tag="mmps")
                    for q in range(2):
                        sl = slice(h * PS + q * 512, h * PS + (q + 1) * 512)
                        nc.tensor.matmul(
                            pt[:O, q * 512 : (q + 1) * 512],
                            lhsT=w4_s, rhs=x3[:, sl], start=True, stop=True,
                        )
                    ot = roll2.tile([O, PS], F32, tag="r2roll")
                    if (2 * ci + h) % 2 == 0:
                        nc.scalar.activation(
                            ot, pt[:O, :], AFT.Identity, bias=b4_sb, scale=1.0
                        )
                    else:
                        nc.vector.tensor_scalar(
                            ot, pt[:O, :], b4_sb, 0.0, ALU.add, ALU.add
                        )
                    nc.sync.dma_start(
                        out=yout[:, cs + h * PS : cs + (h + 1) * PS], in_=ot
                    )

    _split_multi_waits(nc)
    return nc


_NC_CACHE = None


def kernel(**inputs):
    global _NC_CACHE
    pl = np.asarray(inputs["polylines"], np.float32).reshape(BA, T, C)
    W0 = np.asarray(inputs["W0"], np.float32)
    W1 = np.asarray(inputs["W1"], np.float32)
    W2 = np.asarray(inputs["W2"], np.float32)
    W3 = np.asarray(inputs["W3"], np.float32)
    W4 = np.asarray(inputs["W4"], np.float32)
    b4v = np.asarray(inputs["b4"], np.float32)
    g = [np.asarray(inputs[f"g{k}"], np.float32) for k in range(4)]
    be = [np.asarray(inputs[f"be{k}"], np.float32) for k in range(4)]

    # host-side sign folding: W_k columns *= sign(g_k) so s_k >= 0 on device
    sg = [np.where(gk < 0, -1.0, 1.0).astype(np.float32) for gk in g]
    W0f = W0 * sg[0][None, :]
    W1tf = (W1[:H] * sg[1][None, :]).astype(BF16)
    W1bf = (W1[H:] * sg[1][None, :]).astype(BF16)
    W2f = (W2 * sg[2][None, :]).astype(BF16)
    W3f = (W3 * sg[3][None, :]).astype(BF16)
    W4f = W4.astype(BF16)
    gabs_np = np.stack([np.abs(gk) for gk in g], 1).astype(np.float32)  # [H,4]
    bes_np = np.stack(be, 1).astype(np.float32)                        # [H,4]

    shared = {
        "w0": W0f,
        "w1t": np.ascontiguousarray(W1tf),
        "w1b": np.ascontiguousarray(W1bf),
        "w2": np.ascontiguousarray(W2f),
        "w3": np.ascontiguousarray(W3f),
        "w4": np.ascontiguousarray(W4f),
        "gabs": gabs_np,
        "bes": bes_np,
        "b4": b4v.reshape(O, 1),
    }
    in_maps = []
    for i in range(N_CORES):
        rows = pl[i * P_CORE : (i + 1) * P_CORE].reshape(R, C)
        xfm = np.ascontiguousarray(rows.T)  # [C, R] feature-major
        in_maps.append({"xfm": xfm, **shared})

    if _NC_CACHE is None:
        _NC_CACHE = build_nc()
    res = run_bass_kernel_spmd(_NC_CACHE, in_maps, list(range(N_CORES)))

    outs = []
    for i in range(N_CORES):
        o = np.asarray(res.results[i]["out"])  # [O, R] feature-major
        outs.append(o.T.reshape(P_CORE, T, O))
    full = np.concatenate(outs, 0)  # [BA, T, O]
    return full.reshape(B, A, T, O).astype(np.float32)


# revision 15
# speedup vs baseline: 1.1830x; 1.0616x over previous
"""CausalPointNetEncoder on 8 TRN2 NeuronCores (Bass/Tile, SPMD data-parallel).

Strategy
--------
Shard the 1024 polylines 128-per-core. Activations are kept feature-major
([feat partitions, rows free]) so chained matmuls need no transposes
(out_T = W^T @ x_T), BN scale/bias are per-partition ops, and the causal
cummax runs along the free dim via the DVE tensor_tensor_scan instruction
(segmented across polyline boundaries with a mask operand).

BatchNorm uses training-mode batch stats over ALL 262144 rows, so each
layer needs one tiny cross-core AllReduce of [128,2] partial sums
(sum, sum-of-squares). Per-core stats are NOT acceptable (8.7% error).

Algebraic folds used:
 - b0..b3 cancel inside BN (mean subtraction absorbs them).
 - sign(g_k) is folded into W_k columns host-side so s_k = |g_k|*rstd >= 0.
 - x_k = s_k * max(raw_k + c1_k, 0) with c1_k = be_k/s_k - mean(raw_k);
   the s_k scale is folded into the NEXT layer's weight rows on device,
   so the BN apply is a single 2-op DVE tensor_scalar (add, max) at 4x.
 - cummax commutes with (+c, max0, *s>=0): pooled = s*max(cummax(raw)+c1,0),
   so layer 2 needs no separate evacuation pass (the scan reads PSUM).

Host does layout only: transpose input to feature-major, transpose output
back to row-major, dtype conversion of weights to bf16.
"""

import numpy as np

import concourse.bass as bass
import concourse.mybir as mybir
from concourse.tile import TileContext
from concourse.bass_utils import run_bass_kernel_spmd

try:
    import ml_dtypes

    BF16 = np.float16
except ImportError:  # pragma: no cover
    import jax.numpy as jnp

    BF16 = jnp.bfloat16

# problem sizes (hardcoded per harness contract)
B, A, T, C, H, O = 16, 64, 256, 32, 128, 64
N_CORES = 8
BA = B * A                      # 1024 polylines
P_CORE = BA // N_CORES          # 128 polylines per core
R = P_CORE * T                  # 32768 rows per core
N_TOTAL = BA * T                # 262144 rows globally
EPS = 1e-5

CH = 2048                       # rows per chunk (8 polylines)
NCH = R // CH                   # 16 chunks
PS = 1024                       # psum tile free size (2 banks)
NEG = -60000.0  # fp16-safe segment sentinel

F32 = mybir.dt.float32
F32R = mybir.dt.float32r
BF = mybir.dt.float16
ALU = mybir.AluOpType
AFT = mybir.ActivationFunctionType


def _split_multi_waits(nc):
    """This walrus build accepts at most ONE sync wait per instruction.
    Hoist all but the last wait of each instruction onto same-engine NoOp
    carriers inserted immediately before it (engine queues execute in
    program order, so ordering is preserved)."""
    cnt = 0
    for f in nc.m.functions:
        for bb in f.blocks:
            il = bb.instructions
            if not any(i.sync_info and len(i.sync_info.on_wait) > 1 for i in il):
                continue
            new = []
            for inst in il:
                si = inst.sync_info
                waits = list(si.on_wait) if si else []
                if len(waits) > 1:
                    for w in waits[:-1]:
                        nop = mybir.InstNoOp(name=f"I-wsplit-{cnt}", ins=[], outs=[])
                        cnt += 1
                        nop.engine = inst.engine
                        nop.sync_info = mybir.SyncInfo(on_wait=[w], on_update=[])
                        new.append(nop)
                    inst.sync_info = mybir.SyncInfo(
                        on_wait=[waits[-1]], on_update=list(si.on_update)
                    )
                new.append(inst)
            bb.instructions = new
    return cnt


def build_nc():
    nc = bass.Bass()

    xin = nc.declare_dram_parameter("xfm", [C, R], F32R, isOutput=False)
    w0 = nc.declare_dram_parameter("w0", [C, H], F32R, isOutput=False)
    w1t = nc.declare_dram_parameter("w1t", [H, H], BF, isOutput=False)
    w1b = nc.declare_dram_parameter("w1b", [H, H], BF, isOutput=False)
    w2 = nc.declare_dram_parameter("w2", [H, H], BF, isOutput=False)
    w3 = nc.declare_dram_parameter("w3", [H, H], BF, isOutput=False)
    w4 = nc.declare_dram_parameter("w4", [H, O], BF, isOutput=False)
    gabs = nc.declare_dram_parameter("gabs", [H, 4], F32, isOutput=False)
    bes = nc.declare_dram_parameter("bes", [H, 4], F32, isOutput=False)
    b4 = nc.declare_dram_parameter("b4", [O, 1], F32, isOutput=False)
    yout = nc.declare_dram_parameter("out", [O, R], F32, isOutput=True)

    # collective bounce buffers (internal DRAM)
    parts = [nc.dram_tensor(f"part{k}", [H, 2], F32) for k in range(4)]
    reds = [nc.dram_tensor(f"red{k}", [H, 2], F32) for k in range(4)]
    warm_p = nc.dram_tensor("warm_p", [H, 2], F32)
    warm_r = nc.dram_tensor("warm_r", [H, 2], F32)

    with TileContext(nc) as tc:
        with (
            tc.tile_pool(name="sing", bufs=1) as sing,
            tc.tile_pool(name="big", bufs=1) as big,
            tc.tile_pool(name="inp", bufs=3) as inp,
            tc.tile_pool(name="roll", bufs=4) as roll,
            tc.tile_pool(name="roll2", bufs=4) as roll2,
            tc.tile_pool(name="scr", bufs=2) as scrp,
            tc.tile_pool(name="stat", bufs=1) as stat,
            tc.tile_pool(name="ps", bufs=3, space="PSUM") as psp,
            tc.tile_pool(name="pss", bufs=1, space="PSUM") as pss,
        ):
            # ---- weights / constants ----
            w0_sb = sing.tile([C, H], F32R, tag="w0")
            nc.sync.dma_start(out=w0_sb, in_=w0[:])
            w1t_sb = sing.tile([H, H], BF, tag="w1t")
            nc.sync.dma_start(out=w1t_sb, in_=w1t[:])
            w1b_sb = sing.tile([H, H], BF, tag="w1b")
            nc.sync.dma_start(out=w1b_sb, in_=w1b[:])
            w2_sb = sing.tile([H, H], BF, tag="w2")
            nc.sync.dma_start(out=w2_sb, in_=w2[:])
            w3_sb = sing.tile([H, H], BF, tag="w3")
            nc.sync.dma_start(out=w3_sb, in_=w3[:])
            w4_sb = sing.tile([H, O], BF, tag="w4")
            nc.sync.dma_start(out=w4_sb, in_=w4[:])
            gabs_sb = sing.tile([H, 4], F32, tag="gabs")
            nc.sync.dma_start(out=gabs_sb, in_=gabs[:])
            bes_sb = sing.tile([H, 4], F32, tag="bes")
            nc.sync.dma_start(out=bes_sb, in_=bes[:])
            b4_sb = sing.tile([O, 1], F32, tag="b4")
            nc.sync.dma_start(out=b4_sb, in_=b4[:])

            # scaled weight copies (filled after each barrier)
            w1t_s = sing.tile([H, H], BF, tag="w1t_s")
            w1b_s = sing.tile([H, H], BF, tag="w1b_s")
            w2_s = sing.tile([H, H], BF, tag="w2_s")
            w3_s = sing.tile([H, H], BF, tag="w3_s")
            w4_s = sing.tile([H, O], BF, tag="w4_s")

            # scan masks
            mask01 = sing.tile([H, CH], BF, tag="mask01")
            nc.vector.memset(mask01, 1.0)
            nc.vector.memset(
                mask01.rearrange("p (n t) -> p n t", t=T)[:, :, 0:1], 0.0
            )
            maskneg = sing.tile([H, PS], BF, tag="maskneg")
            nc.vector.memset(maskneg, 0.0)
            nc.vector.memset(
                maskneg.rearrange("p (n t) -> p n t", t=T)[:, :, 0:1], NEG
            )

            # warmup collective: pays the cold CC handshake during phase 0
            wtile = stat.tile([H, 2], F32, tag="wtile")
            nc.vector.memset(wtile, 0.0)
            nc.sync.dma_start(out=warm_p[:], in_=wtile)
            nc.gpsimd.collective_compute(
                "AllReduce", ALU.add,
                replica_groups=[list(range(N_CORES))],
                ins=[warm_p[:]], outs=[warm_r[:]],
            )

            # staged activations (bf16): A and Cbuf are reused across layers
            preA = big.tile([H, R], BF, tag="bigA")   # pre0; later cmax2
            preC = big.tile([H, R], BF, tag="bigC")   # pre1; later pre3

            # accumulators: one column per accum op instance
            accS = [stat.tile([H, 2 * NCH], F32, name=f"accS{k}", tag=f"accS{k}") for k in range(4)]
            accQ = [stat.tile([H, 2 * NCH], F32, name=f"accQ{k}", tag=f"accQ{k}") for k in range(4)]
            for t_ in accS + accQ:
                nc.vector.memset(t_, 0.0)

            # per-layer stat vectors
            c1 = [stat.tile([H, 1], F32, name=f"c1_{k}", tag=f"c1_{k}") for k in range(4)]
            svec = [stat.tile([H, 1], F32, name=f"s_{k}", tag=f"s_{k}") for k in range(4)]
            tmp1 = stat.tile([H, 1], F32, tag="tmp1")
            tmp2 = stat.tile([H, 1], F32, tag="tmp2")
            tmp3 = stat.tile([H, 1], F32, tag="tmp3")
            gpart = stat.tile([H, 2], F32, tag="gpart")
            eps_sb = stat.tile([H, 1], F32, tag="eps")
            nc.vector.memset(eps_sb, EPS)
            gstat = [stat.tile([H, 2], F32, name=f"gstat{k}", tag=f"gstat{k}") for k in range(4)]

            def barrier(k, w_scale_jobs):
                """Reduce local accums -> allreduce -> finalize s_k, c1_k,
                scale next-layer weights."""
                nc.vector.tensor_reduce(
                    gpart[:, 0:1], accS[k][:], mybir.AxisListType.X, ALU.add
                )
                nc.vector.tensor_reduce(
                    gpart[:, 1:2], accQ[k][:], mybir.AxisListType.X, ALU.add
                )
                nc.sync.dma_start(out=parts[k][:], in_=gpart)
                nc.gpsimd.collective_compute(
                    "AllReduce",
                    ALU.add,
                    replica_groups=[list(range(N_CORES))],
                    ins=[parts[k][:]],
                    outs=[reds[k][:]],
                )
                nc.sync.dma_start(out=gstat[k], in_=reds[k][:])
                g = gstat[k]
                sumv, sumq = g[:, 0:1], g[:, 1:2]
                if k == 2:
                    # sum(raw2) = W2s^T @ sum(x1~)  (tiny fp32 matmul, FD=1)
                    w2_sf = stat.tile([H, H], F32, tag="w2_sf")
                    nc.vector.tensor_copy(w2_sf, w2_s)
                    ps1 = pss.tile([H, 1], F32, tag="stats_ps")
                    nc.tensor.matmul(ps1, lhsT=w2_sf, rhs=sumv, start=True, stop=True)
                    nc.vector.tensor_copy(tmp3, ps1)
                    sumv = tmp3
                # mu = sum/N ; m2 = sumsq/N ; var = m2 - mu^2
                mu = tmp1
                nc.vector.tensor_scalar(mu, sumv, 1.0 / N_TOTAL, None, ALU.mult)
                nc.vector.tensor_scalar(tmp2, sumq, 1.0 / N_TOTAL, None, ALU.mult)
                var = tmp2
                musq = stat.tile([H, 1], F32, tag="musq")
                nc.vector.tensor_tensor(musq, mu, mu, ALU.mult)
                nc.vector.tensor_tensor(var, var, musq, ALU.subtract)
                # s = |g| / sqrt(var + eps)
                std = stat.tile([H, 1], F32, tag="std")
                nc.scalar.activation(std, var, AFT.Sqrt, bias=eps_sb, scale=1.0)
                rstd = stat.tile([H, 1], F32, tag="rstd")
                nc.vector.reciprocal(rstd, std)
                nc.vector.tensor_tensor(svec[k], rstd, gabs_sb[:, k : k + 1], ALU.mult)
                nc.vector.tensor_scalar(svec[k], svec[k], 1e-20, None, ALU.max)
                # c1 = be/s - mu
                recs = stat.tile([H, 1], F32, tag="recs")
                nc.vector.reciprocal(recs, svec[k])
                nc.vector.tensor_tensor(c1[k], bes_sb[:, k : k + 1], recs, ALU.mult)
                nc.vector.tensor_tensor(c1[k], c1[k], mu, ALU.subtract)
                # fold s_k into next-layer weight rows
                for wdst, wsrc in w_scale_jobs:
                    nc.vector.tensor_scalar(wdst, wsrc, svec[k], None, ALU.mult)

            # ================= phase 0: mm0 + evac0 + sumsq0 ==============
            for ci in range(NCH):
                cs = ci * CH
                xt = inp.tile([C, CH], F32R, tag="xin")
                nc.sync.dma_start(out=xt, in_=xin[:, cs : cs + CH])
                for h in range(2):
                    pt = psp.tile([H, PS], F32, tag="mmps")
                    for q in range(2):
                        nc.tensor.matmul(
                            pt[:, q * 512 : (q + 1) * 512],
                            lhsT=w0_sb,
                            rhs=xt[:, h * PS + q * 512 : h * PS + (q + 1) * 512],
                            start=True,
                            stop=True,
                        )
                    dst = preA[:, cs + h * PS : cs + (h + 1) * PS]
                    nc.scalar.activation(
                        dst, pt, AFT.Copy, accum_out=accS[0][:, 2 * ci + h : 2 * ci + h + 1]
                    )
                    scr = scrp.tile([H, PS], BF, tag="scr")
                    nc.vector.scalar_tensor_tensor(
                        scr, pt, 1.0, dst, ALU.mult, ALU.mult,
                        accum_out=accQ[0][:, 2 * ci + h : 2 * ci + h + 1],
                    )
            barrier(0, [(w1t_s, w1t_sb), (w1b_s, w1b_sb)])

            # ============ phase 1: apply0 + scan0 + mm1 + evac1/sumsq1 ====
            for ci in range(NCH):
                cs = ci * CH
                x0 = roll.tile([H, CH], BF, tag="xroll")
                nc.vector.tensor_scalar(
                    x0, preA[:, cs : cs + CH], c1[0], 0.0, ALU.add, ALU.max
                )
                p0 = roll2.tile([H, CH], BF, tag="r2roll")
                nc.vector.tensor_tensor_scan(
                    p0, mask01, x0, 0.0, ALU.mult, ALU.max
                )
                for h in range(2):
                    pt = psp.tile([H, PS], F32, tag="mmps")
                    for q in range(2):
                        sl = slice(h * PS + q * 512, h * PS + (q + 1) * 512)
                        nc.tensor.matmul(
                            pt[:, q * 512 : (q + 1) * 512],
                            lhsT=w1t_s, rhs=x0[:, sl], start=True, stop=False,
                        )
                        nc.tensor.matmul(
                            pt[:, q * 512 : (q + 1) * 512],
                            lhsT=w1b_s, rhs=p0[:, sl], start=False, stop=True,
                        )
                    dst = preC[:, cs + h * PS : cs + (h + 1) * PS]
                    nc.scalar.activation(
                        dst, pt, AFT.Copy, accum_out=accS[1][:, 2 * ci + h : 2 * ci + h + 1]
                    )
                    scr = scrp.tile([H, PS], BF, tag="scr")
                    nc.scalar.activation(
                        scr, pt, AFT.Square,
                        accum_out=accQ[1][:, 2 * ci + h : 2 * ci + h + 1],
                    )
            barrier(1, [(w2_s, w2_sb)])

            # ============ phase 2: apply1 + mm2 + scan2(PSUM) + sumsq2 ====
            for ci in range(NCH):
                cs = ci * CH
                x1 = roll.tile([H, CH], BF, tag="xroll")
                # NB: tensor_scalar+accum_out is broken on this HW path
                # (probe: accum garbage AND corrupt main out) — use ACT Relu.
                nc.scalar.activation(
                    x1, preC[:, cs : cs + CH], AFT.Relu, bias=c1[1], scale=1.0,
                    accum_out=accS[2][:, 2 * ci : 2 * ci + 1],
                )
                for h in range(2):
                    pt = psp.tile([H, PS], F32, tag="mmps")
                    for q in range(2):
                        sl = slice(h * PS + q * 512, h * PS + (q + 1) * 512)
                        nc.tensor.matmul(
                            pt[:, q * 512 : (q + 1) * 512],
                            lhsT=w2_s, rhs=x1[:, sl], start=True, stop=True,
                        )
                    # scan evacuates PSUM -> cmax2 (reuses preA storage)
                    nc.vector.tensor_tensor_scan(
                        preA[:, cs + h * PS : cs + (h + 1) * PS],
                        maskneg, pt, NEG, ALU.add, ALU.max,
                    )
                    scr2 = scrp.tile([H, PS], BF, tag="scr")
                    nc.scalar.activation(
                        scr2,
                        pt,
                        AFT.Square,
                        accum_out=accQ[2][:, 2 * ci + h : 2 * ci + h + 1],
                    )
            barrier(2, [(w3_s, w3_sb)])

            # ============ phase 3: p2-apply + mm3 + evac3/sumsq3 ==========
            for ci in range(NCH):
                cs = ci * CH
                p2 = roll.tile([H, CH], BF, tag="xroll")
                nc.vector.tensor_scalar(
                    p2, preA[:, cs : cs + CH], c1[2], 0.0, ALU.add, ALU.max
                )
                for h in range(2):
                    pt = psp.tile([H, PS], F32, tag="mmps")
                    for q in range(2):
                        sl = slice(h * PS + q * 512, h * PS + (q + 1) * 512)
                        nc.tensor.matmul(
                            pt[:, q * 512 : (q + 1) * 512],
                            lhsT=w3_s, rhs=p2[:, sl], start=True, stop=True,
                        )
                    dst = preC[:, cs + h * PS : cs + (h + 1) * PS]
                    nc.scalar.activation(
                        dst, pt, AFT.Copy, accum_out=accS[3][:, 2 * ci + h : 2 * ci + h + 1]
                    )
                    scr = scrp.tile([H, PS], BF, tag="scr")
                    if (2 * ci + h) % 2 == 0:
                        nc.vector.scalar_tensor_tensor(
                            scr, pt, 1.0, dst, ALU.mult, ALU.mult,
                            accum_out=accQ[3][:, 2 * ci + h : 2 * ci + h + 1],
                        )
                    else:
                        nc.scalar.activation(
                            scr, pt, AFT.Square,
                            accum_out=accQ[3][:, 2 * ci + h : 2 * ci + h + 1],
                        )
            barrier(3, [(w4_s, w4_sb)])

            # ============ phase 4: apply3 + mm4 + bias + out ==============
            for ci in range(NCH):
                cs = ci * CH
                x3 = roll.tile([H, CH], BF, tag="xroll")
                nc.vector.tensor_scalar(
                    x3, preC[:, cs : cs + CH], c1[3], 0.0, ALU.add, ALU.max
                )
                for h in range(2):
                    pt = psp.tile([H, PS], F32, tag="mmps")
                    for q in range(2):
                        sl = slice(h * PS + q * 512, h * PS + (q + 1) * 512)
                        nc.tensor.matmul(
                            pt[:O, q * 512 : (q + 1) * 512],
                            lhsT=w4_s, rhs=x3[:, sl], start=True, stop=True,
                        )
                    ot = roll2.tile([O, PS], F32, tag="r2roll")
                    if (2 * ci + h) % 2 == 0:
                        nc.scalar.activation(
                            ot, pt[:O, :], AFT.Identity, bias=b4_sb, scale=1.0
                        )
                    else:
                        nc.vector.tensor_scalar(
                            ot, pt[:O, :], b4_sb, 0.0, ALU.add, ALU.add
                        )
                    nc.sync.dma_start(
                        out=yout[:, cs + h * PS : cs + (h + 1) * PS], in_=ot
                    )

    _split_multi_waits(nc)
    return nc


_NC_CACHE = None


def kernel(**inputs):
    global _NC_CACHE
    pl = np.asarray(inputs["polylines"], np.float32).reshape(BA, T, C)
    W0 = np.asarray(inputs["W0"], np.float32)
    W1 = np.asarray(inputs["W1"], np.float32)
    W2 = np.asarray(inputs["W2"], np.float32)
    W3 = np.asarray(inputs["W3"], np.float32)
    W4 = np.asarray(inputs["W4"], np.float32)
    b4v = np.asarray(inputs["b4"], np.float32)
    g = [np.asarray(inputs[f"g{k}"], np.float32) for k in range(4)]
    be = [np.asarray(inputs[f"be{k}"], np.float32) for k in range(4)]

    # host-side sign folding: W_k columns *= sign(g_k) so s_k >= 0 on device
    sg = [np.where(gk < 0, -1.0, 1.0).astype(np.float32) for gk in g]
    W0f = W0 * sg[0][None, :]
    W1tf = (W1[:H] * sg[1][None, :]).astype(BF16)
    W1bf = (W1[H:] * sg[1][None, :]).astype(BF16)
    W2f = (W2 * sg[2][None, :]).astype(BF16)
    W3f = (W3 * sg[3][None, :]).astype(BF16)
    W4f = W4.astype(BF16)
    gabs_np = np.stack([np.abs(gk) for gk in g], 1).astype(np.float32)  # [H,4]
    bes_np = np.stack(be, 1).astype(np.float32)                        # [H,4]

    shared = {
        "w0": W0f,
        "w1t": np.ascontiguousarray(W1tf),
        "w1b": np.ascontiguousarray(W1bf),
        "w2": np.ascontiguousarray(W2f),
        "w3": np.ascontiguousarray(W3f),
        "w4": np.ascontiguousarray(W4f),
        "gabs": gabs_np,
        "bes": bes_np,
        "b4": b4v.reshape(O, 1),
    }
    in_maps = []
    for i in range(N_CORES):
        rows = pl[i * P_CORE : (i + 1) * P_CORE].reshape(R, C)
        xfm = np.ascontiguousarray(rows.T)  # [C, R] feature-major
        in_maps.append({"xfm": xfm, **shared})

    if _NC_CACHE is None:
        _NC_CACHE = build_nc()
    res = run_bass_kernel_spmd(_NC_CACHE, in_maps, list(range(N_CORES)))

    outs = []
    for i in range(N_CORES):
        o = np.asarray(res.results[i]["out"])  # [O, R] feature-major
        outs.append(o.T.reshape(P_CORE, T, O))
    full = np.concatenate(outs, 0)  # [BA, T, O]
    return full.reshape(B, A, T, O).astype(np.float32)
